# revision 8
# baseline (speedup 1.0000x reference)
"""Trainium2 Bass kernel for an involution Bottleneck block (B=2, Cin=256,
Cmid=64, Cout=256, H=W=56, K=15, G=4).

Sharding: 8 cores = 2 batches x 4 H-quarters (14 output rows each). Each core
receives a zero-padded input halo [256, 28, 70] bf16; no inter-core traffic.

v2 pipeline changes vs v1 baseline (147us):
 - input DMA split into 4 column chunks, ordered (c1,c2,c0,c3) so conv1 /
   reduce / span start ~15us earlier; wse split into 3 chunk DMAs on the
   DVE queue; residual is read from the padded bf16 input slab (the
   separate fp32 residual input is gone: -800KB of head DMA).
 - per-PAIR span matmuls (512+272 within a [128,1024] 2-bank PSUM tile,
   bufs=3) with weight chunks rotating across partition bands 0/32/64 so
   LDWEIGHTS overlaps compute; one ACT eviction per pair.
 - involution MAC split across engines: DVE does most pairs (multiply +
   16-wide add trees); GpSimd (idle in v1) takes every 7th pair with its
   own f32 running accumulator.
 - dup/replication DMAs moved to the GpSimd software DGE; out1 shift
   copies emitted mid-stream so nothing blocks the first multiplies.
"""

import sys
sys.path.insert(0, "/opt/trn_rl_repo")

import numpy as np
import ml_dtypes
from contextlib import ExitStack

import concourse.bass as bass
import concourse.mybir as mybir
import concourse.tile as tile
from concourse import bacc
from concourse.bass import ts
from concourse.bass_utils import run_bass_kernel_spmd

BF16 = mybir.dt.bfloat16
F32 = mybir.dt.float32
AF = mybir.ActivationFunctionType

K = 15
G = 4
GC = 16
PAD = 7
CIN = 256
CMID = 64
RED = 16
COUT = 256
H = 56
W = 56
B = 2
HB = 14            # output rows per core
HP = HB + 2 * PAD  # 28 padded rows
WP = W + 2 * PAD   # 70 padded cols
NP = HP * WP       # 1960
NPX = HB * W       # 784 output pixels per core
DUPW = (HP - PAD) * WP  # 1470: dup rows 0..20 (orig rows 7..27)


def _pair_list():
    """113 tap-pairs: (kind, ky, kx). 'kk' pairs taps (ky,kx)+(ky+7,kx);
    'r14' pairs (14,kx)+(14,kx+1) (solo for kx=14). Even kx first per ky so
    the odd-kx shifted copy (out1q) is not needed until ~pair 8."""
    pairs = []
    for ky in range(7):
        for kx in range(0, K, 2):
            pairs.append(("kk", ky, kx))
        for kx in range(1, K, 2):
            pairs.append(("kk", ky, kx))
    for j in range(8):
        pairs.append(("r14", 14, 2 * j))
    return pairs


PAIRS = _pair_list()
NPAIR = len(PAIRS)  # 113


def _chunk_map():
    """pair -> (wse partition-band chunk 0/1/2, column slot). First 6 pairs on
    chunk 0 so band-32/64 r replication has time to land."""
    m = {}
    cols = [0, 0, 0]
    for pi in range(NPAIR):
        c = 0 if pi < 12 else pi % 3
        m[pi] = (c, cols[c])
        cols[c] += 1
    return m, cols


CHUNK_MAP, CHUNK_COLS = _chunk_map()
WMAX = max(CHUNK_COLS)  # 46

GP_BLOCKS = {1}  # 16-pair product blocks whose add-tree runs on GpSimd

_PROGRAM = None


def _build_program():
    nc = bacc.Bacc(None, target_bir_lowering=False, debug=False)
    with tile.TileContext(nc) as tc, ExitStack() as ctx:
        dram = ctx.enter_context(tc.tile_pool(name="dram", bufs=1, space="DRAM"))
        xb_d = dram.tile([4 * 128, 980], BF16, kind="ExternalInput", name="xb")
        w1t_d = dram.tile([CIN, CMID], BF16, kind="ExternalInput", name="w1t")
        wrt_d = dram.tile([CMID, RED], BF16, kind="ExternalInput", name="wrt")
        wse_d = dram.tile([81, WMAX * 128], BF16, kind="ExternalInput", name="wse")
        w3t_d = dram.tile([CMID, COUT], BF16, kind="ExternalInput", name="w3t")
        vec_d = dram.tile([128, 10], F32, kind="ExternalInput", name="vecs")
        ones_d = dram.tile([1, NPX], BF16, kind="ExternalInput", name="ones")
        y_d = dram.tile([COUT, HB * W], F32, kind="ExternalOutput", name="y")

        xpool = ctx.enter_context(tc.tile_pool(name="xin", bufs=1))
        xb = xpool.tile([128, 2, NP], BF16)
        # order (1,2,0,3): chunks 1,2 feed the reduce; 0 feeds the first
        # multiplies; 3 is only needed for the dup upper rows
        for j in (1, 2, 0, 3):
            nc.sync.dma_start(
                out=xb[:, :, ts(j, 490)],
                in_=xb_d[128 * j:128 * (j + 1), :].rearrange("p (c n) -> p c n", c=2))

        wpool = ctx.enter_context(tc.tile_pool(name="weights", bufs=1))
        w1t = wpool.tile([128, 2, CMID], BF16)
        nc.scalar.dma_start(out=w1t[:], in_=w1t_d[:].rearrange("(c p) m -> p c m", p=128))
        vecs = wpool.tile([128, 10], F32)
        nc.scalar.dma_start(out=vecs[:], in_=vec_d[:])
        wrt = wpool.tile([CMID, RED], BF16)
        nc.scalar.dma_start(out=wrt[:], in_=wrt_d[:])
        w3t = wpool.tile([CMID, COUT], BF16)

        wse = wpool.tile([81, WMAX * 128], BF16)
        spool = ctx.enter_context(tc.tile_pool(name="stage", bufs=1))
        r_sb = spool.tile([81, NPX], BF16)
        # wse chunk 0 gates the first span matmuls: pairs 0-11 (cols 0:12)
        # first on the ACT queue, the rest behind; chunks 1/2 + w3t via SWDGE
        nc.scalar.dma_start(out=wse[0:RED + 1, 0:12 * 128],
                            in_=wse_d[0:RED + 1, 0:12 * 128])
        nc.scalar.dma_start(out=wse[0:RED + 1, 12 * 128:CHUNK_COLS[0] * 128],
                            in_=wse_d[0:RED + 1, 12 * 128:CHUNK_COLS[0] * 128])
        nc.gpsimd.dma_start(out=r_sb[RED:RED + 1, :], in_=ones_d[:])
        for c in (1, 2):
            nc.gpsimd.dma_start(out=wse[32 * c:32 * c + RED + 1, 0:CHUNK_COLS[c] * 128],
                                in_=wse_d[32 * c:32 * c + RED + 1, 0:CHUNK_COLS[c] * 128])

        opool = ctx.enter_context(tc.tile_pool(name="out1", bufs=1))
        out1p = opool.tile([128, NP], BF16)
        out1q = opool.tile([128, NP], BF16)
        out1r = opool.tile([128, NP], BF16)
        o3 = out1p[:].rearrange("p (h w) -> p h w", w=WP)
        o3q = out1q[:].rearrange("p (h w) -> p h w", w=WP)
        o3r = out1r[:].rearrange("p (h w) -> p h w", w=WP)

        nc.gpsimd.memset(out1r[CMID:128, NP - 1:NP], 0.0)

        # conv1 + reduce, interleaved so the reduce runs as soon as its rows land
        with tc.tile_pool(name="p1", bufs=4, space="PSUM") as p1, \
             tc.tile_pool(name="pr", bufs=2, space="PSUM") as pr:
            def conv1_chunk(j):
                ps = p1.tile([CMID, 490], F32, tag="ps1")
                nc.tensor.matmul(ps[:], w1t[:, 0, :], xb[:, 0, ts(j, 490)],
                                 start=True, stop=False)
                nc.tensor.matmul(ps[:], w1t[:, 1, :], xb[:, 1, ts(j, 490)],
                                 start=False, stop=True)
                nc.scalar.activation(out1p[0:CMID, ts(j, 490)], ps[:], AF.Relu,
                                     bias=vecs[0:CMID, 1:2], scale=vecs[0:CMID, 0:1])

            def reduce_half(hh):
                ps = pr.tile([RED, 392], F32, tag="psr")
                nc.tensor.matmul(ps[:], wrt[:],
                                 o3[0:CMID, PAD + 7 * hh:PAD + 7 * (hh + 1), PAD:PAD + W],
                                 start=True, stop=True)
                nc.scalar.activation(r_sb[0:RED, ts(hh, 392)], ps[:], AF.Relu,
                                     bias=vecs[0:RED, 3:4], scale=vecs[0:RED, 2:3])

            conv1_chunk(1)
            conv1_chunk(2)
            # dup rows 0..13 (orig 7..20): readable once chunks 1,2 evicted
            nc.gpsimd.dma_start(out=out1p[CMID:128, 0:980],
                                in_=out1p[0:CMID, 490:1470])
            reduce_half(0)
            reduce_half(1)
            # r replication to partition bands 32/64 for span chunks 1/2
            nc.gpsimd.dma_start(out=r_sb[32:32 + RED + 1, :], in_=r_sb[0:RED + 1, :])
            nc.gpsimd.dma_start(out=r_sb[64:64 + RED + 1, :], in_=r_sb[0:RED + 1, :])
            conv1_chunk(0)
            conv1_chunk(3)
            # dup rows 14..20 (orig 21..27): needs chunk 3
            nc.gpsimd.dma_start(out=out1p[CMID:128, 980:DUPW],
                                in_=out1p[0:CMID, 1470:NP])
            nc.gpsimd.dma_start(out=w3t[:], in_=w3t_d[:])

        # involution: 2-pair span tiles -> one ACT evict per tile -> DVE
        # multiplies; the add-tree for blocks in GP_BLOCKS runs on GpSimd
        acc = None          # DVE accumulator [128, NPX] bf16
        gsums = []          # per-GP-block sums
        dp = 0

        def tree_collapse(nc_eng, pool, prod_t, n, tags):
            cur, cnt = prod_t, n
            while cnt > 1:
                h = cnt // 2
                dst = pool.tile([128, h, NPX], BF16, tag=f"{tags}{h}", bufs=1)
                nc_eng.tensor_add(dst[:], cur[:, 0:h, :], cur[:, h:2 * h, :])
                if cnt % 2:
                    d0 = pool.tile([128, h, NPX], BF16, tag=f"{tags}{h}", bufs=1)
                    nc_eng.tensor_add(d0[:, 0, :], dst[:, 0, :], cur[:, cnt - 1, :])
                    if h > 1:
                        nc_eng.tensor_copy(d0[:, 1:h, :], dst[:, 1:h, :])
                    dst = d0
                cur, cnt = dst, h
            return cur[:, 0, :] if cur.shape[1] > 1 or n == 1 else cur[:, 0, :]

        with tc.tile_pool(name="sp", bufs=2, space="PSUM") as sp, \
             tc.tile_pool(name="we", bufs=5) as we_pool, \
             tc.tile_pool(name="prod", bufs=3) as prod_pool, \
             tc.tile_pool(name="accp", bufs=2) as acc_pool, \
             tc.tile_pool(name="gpp", bufs=2) as gp_pool:
            prod = None
            NT = (NPAIR + 1) // 2
            for t in range(NT):
                plist = [p for p in (2 * t, 2 * t + 1) if p < NPAIR]
                ps = sp.tile([128, 2048], F32, tag="spanps")
                for i, pi in enumerate(plist):
                    c, col = CHUNK_MAP[pi]
                    lhsT = wse[32 * c:32 * c + RED + 1, ts(col, 128)]
                    rr = r_sb[32 * c:32 * c + RED + 1, :]
                    nc.tensor.matmul(ps[:, 1024 * i:1024 * i + 512], lhsT,
                                     rr[:, 0:512], start=True, stop=True)
                    nc.tensor.matmul(ps[:, 1024 * i + 512:1024 * i + NPX], lhsT,
                                     rr[:, 512:NPX], start=True, stop=True)
                we = we_pool.tile([128, len(plist), NPX], BF16, tag="we")
                nc.scalar.activation(
                    we[:], ps[:].rearrange("p (q x) -> p q x", x=1024)[:, 0:len(plist), 0:NPX],
                    AF.Copy, scale=1.0)

                for i, pi in enumerate(plist):
                    kind, ky, kx = PAIRS[pi]
                    if kind == "kk":
                        src = o3[:, ky:ky + HB, kx:kx + W] if kx % 2 == 0 else \
                            o3q[:, ky:ky + HB, kx - 1:kx - 1 + W]
                    else:
                        src = o3r[:, 14:14 + HB, kx:kx + W]
                    if dp % 16 == 0:
                        prod = prod_pool.tile([128, 16, NPX], BF16, tag="prod")
                    nc.vector.tensor_mul(
                        prod[:, dp % 16, :].rearrange("p (h w) -> p h w", w=W),
                        we[:, i, :].rearrange("p (h w) -> p h w", w=W), src)
                    dp += 1
                    if dp % 16 == 0 or dp == NPAIR:
                        n = (dp - 1) % 16 + 1
                        blk = (dp - 1) // 16
                        if blk in GP_BLOCKS and n == 16:
                            gs = tree_collapse(nc.gpsimd, gp_pool, prod, n, "g")
                            gsums.append(gs)
                        else:
                            s_ap = tree_collapse(nc.vector, acc_pool, prod, n, "t")
                            if acc is None:
                                na = acc_pool.tile([128, NPX], BF16, tag="acc")
                                nc.vector.tensor_copy(na[:], s_ap)
                            else:
                                na = acc_pool.tile([128, NPX], BF16, tag="acc")
                                nc.vector.tensor_add(na[:], acc[:], s_ap)
                            acc = na

                    # mid-stream shifted copies
                    if pi == 5:
                        nc.vector.tensor_copy(out1q[0:CMID, 0:NP - 1],
                                              out1p[0:CMID, 1:NP])
                        nc.vector.tensor_copy(out1q[CMID:128, 0:DUPW - 1],
                                              out1p[CMID:128, 1:DUPW])
                    if pi == 16:
                        nc.vector.tensor_copy(out1r[0:CMID, :], out1p[0:CMID, :])
                        nc.sync.dma_start(out=out1r[CMID:128, 0:NP - 1],
                                          in_=out1p[0:CMID, 1:NP])
            # combine the GP block sums on GpSimd
            gacc = gsums[0]
            for gs in gsums[1:]:
                gn = gp_pool.tile([128, NPX], BF16, tag="gacc")
                nc.gpsimd.tensor_add(gn[:], gacc[:], gs)
                gacc = gn

        # merge tap-halves + fold in the GP accumulator, then BN2+ReLU
        tmpD = spool.tile([CMID, NPX], BF16)
        nc.sync.dma_start(out=tmpD[:], in_=acc[CMID:128, :])
        tmpG = spool.tile([CMID, NPX], BF16)
        nc.sync.dma_start(out=tmpG[:], in_=gacc[CMID:128, :])
        out2f = spool.tile([CMID, NPX], BF16)
        mpool = ctx.enter_context(tc.tile_pool(name="merge", bufs=2))
        for h in range(2):
            a1 = mpool.tile([CMID, 392], BF16, tag="a1")
            nc.vector.tensor_add(a1[:], acc[0:CMID, ts(h, 392)], tmpD[:, ts(h, 392)])
            a2 = mpool.tile([CMID, 392], BF16, tag="a2")
            nc.vector.tensor_add(a2[:], gacc[0:CMID, ts(h, 392)], tmpG[:, ts(h, 392)])
            a3 = mpool.tile([CMID, 392], BF16, tag="a3")
            nc.vector.tensor_add(a3[:], a1[:], a2[:])
            nc.scalar.activation(out2f[:, ts(h, 392)], a3[:], AF.Relu,
                                 bias=vecs[0:CMID, 5:6], scale=vecs[0:CMID, 4:5])

        # conv3 + BN3 + residual (from the bf16 input slab) + relu
        xbr = xb[:].rearrange("p c (h w) -> p c h w", w=WP)
        with tc.tile_pool(name="p3", bufs=2, space="PSUM") as p3, \
             tc.tile_pool(name="ypool", bufs=2) as ypool:
            for nh in range(2):
                for mc in range(2):
                    ps = p3.tile([128, 392], F32, tag="ps3")
                    nc.tensor.matmul(ps[:], w3t[:, ts(mc, 128)], out2f[:, ts(nh, 392)],
                                     start=True, stop=True)
                    t3 = ypool.tile([128, 392], F32, tag="t3")
                    nc.scalar.activation(t3[:], ps[:], AF.Identity,
                                         bias=vecs[:, 8 + mc:9 + mc],
                                         scale=vecs[:, 6 + mc:7 + mc])
                    ys = ypool.tile([128, 392], F32, tag="ys")
                    nc.vector.tensor_add(
                        ys[:].rearrange("p (h w) -> p h w", w=W), t3[:].rearrange("p (h w) -> p h w", w=W),
                        xbr[:, mc, PAD + 7 * nh:PAD + 7 * (nh + 1), PAD:PAD + W])
                    yr = ypool.tile([128, 392], F32, tag="yr")
                    nc.scalar.activation(yr[:], ys[:], AF.Relu, scale=1.0)
                    nc.sync.dma_start(
                        out=y_d[:].rearrange("(c p) n -> p c n", p=128)[:, mc, ts(nh, 392)],
                        in_=yr[:])

    nc.compile()
    names = dict(xb=xb_d.name, w1t=w1t_d.name, wrt=wrt_d.name,
                 wse=wse_d.name, w3t=w3t_d.name, vecs=vec_d.name,
                 ones=ones_d.name, y=y_d.name)
    return nc, names


def _get_program():
    global _PROGRAM
    if _PROGRAM is None:
        _PROGRAM = _build_program()
    return _PROGRAM


def _bf16(a):
    return np.asarray(a, dtype=np.float32).astype(ml_dtypes.bfloat16)


def kernel(x, W1, g1, b1, Wr, gr, br, Ws, bs, g2, b2, W3, g3, b3,
           _want_results=False, _trace=False):
    x = np.asarray(x, dtype=np.float32)
    nc, names = _get_program()

    w1t = _bf16(np.asarray(W1).T)                      # [256, 64]
    wrt = _bf16(np.asarray(Wr).T)                      # [64, 16]
    w3t = _bf16(np.asarray(W3).T)                      # [64, 256]

    # span weights, 16x channel-expanded, tap-paired, bias at row 16.
    Ws = np.asarray(Ws, dtype=np.float32)              # [900, 16]
    bs = np.asarray(bs, dtype=np.float32)              # [900]
    gidx = np.arange(CMID) // GC
    WsT = Ws.reshape(G, K * K, RED)
    bsr = bs.reshape(G, K * K)
    wse4 = np.zeros((81, WMAX * 128), dtype=np.float32)
    for pi, (kind, ky, kx) in enumerate(PAIRS):
        if kind == "kk":
            k1, k2 = ky * K + kx, (ky + 7) * K + kx
        else:
            k1 = 14 * K + kx
            k2 = 14 * K + kx + 1 if kx + 1 < K else None
        c, col = CHUNK_MAP[pi]
        blk = np.zeros((RED + 1, 128), dtype=np.float32)
        blk[0:RED, 0:CMID] = WsT[gidx, k1, :].T
        blk[RED, 0:CMID] = bsr[gidx, k1]
        if k2 is not None:
            blk[0:RED, CMID:128] = WsT[gidx, k2, :].T
            blk[RED, CMID:128] = bsr[gidx, k2]
        wse4[32 * c:32 * c + RED + 1, col * 128:(col + 1) * 128] = blk
    wse = _bf16(wse4)

    vecs = np.zeros((128, 10), dtype=np.float32)
    vecs[0:CMID, 0] = g1
    vecs[0:CMID, 1] = b1
    vecs[0:RED, 2] = gr
    vecs[0:RED, 3] = br
    vecs[0:CMID, 4] = g2
    vecs[0:CMID, 5] = b2
    vecs[:, 6] = np.asarray(g3)[0:128]
    vecs[:, 7] = np.asarray(g3)[128:256]
    vecs[:, 8] = np.asarray(b3)[0:128]
    vecs[:, 9] = np.asarray(b3)[128:256]

    in_maps = []
    core_geom = []
    for core in range(8):
        b = core // 4
        h0 = (core % 4) * HB
        xpad = np.zeros((CIN, HP, WP), dtype=np.float32)
        lo, hi = h0 - PAD, h0 + HB + PAD
        slo, shi = max(lo, 0), min(hi, H)
        xpad[:, slo - lo:shi - lo, PAD:PAD + W] = x[b, :, slo:shi, :]
        xflat = _bf16(xpad).reshape(CIN, NP)          # [(c p), (pix)]
        # chunk-major layout [4, 128, 2, 490] -> [512, 980] so each chunk DMA
        # reads 128 contiguous 1960B runs
        xsw = np.empty((4, 128, 2, 490), dtype=ml_dtypes.bfloat16)
        for j in range(4):
            for ci in range(2):
                xsw[j, :, ci, :] = xflat[ci * 128:(ci + 1) * 128, 490 * j:490 * (j + 1)]
        xbc = xsw.reshape(512, 980)
        in_maps.append({
            names["xb"]: xbc,
            names["w1t"]: w1t,
            names["wrt"]: wrt,
            names["wse"]: wse,
            names["w3t"]: w3t,
            names["vecs"]: vecs,
            names["ones"]: np.ones((1, NPX), dtype=np.float32).astype(ml_dtypes.bfloat16),
        })
        core_geom.append((b, h0))

    res = run_bass_kernel_spmd(nc, in_maps, list(range(8)), trace=_trace)

    y = np.empty((B, COUT, H, W), dtype=np.float32)
    for core, (b, h0) in enumerate(core_geom):
        y[b, :, h0:h0 + HB, :] = res.results[core][names["y"]].reshape(COUT, HB, W)
    if _want_results:
        return y, res
    return y
